# revision 15
# baseline (speedup 1.0000x reference)
"""APPNP (3-branch MLP + fused linear propagation) on 8 Trainium2 NeuronCores.

Math: the APPNP propagation is linear in h, so
    a1*P(h1) + a2*P(h2) + a3*P(h3) = P(a1*h1 + a2*h2 + a3*h3).
We compute h0c = sum_i a_i * mlp_i(x_i) once (on device), then run K=10
propagation steps on the single fused [N, 64] tensor.

Distribution: nodes are sharded across 8 cores (balanced by in-degree) and
bin-packed into blocks of 128 "slots" (a permuted, padded node order).  Each
step: AllGather the pre-scaled node states g = norm*h into a full table in
each core's HBM, dma_gather the per-edge source rows (edges grouped by
destination block and source quarter so int16 gather indices stay in range),
segment-sum via one-hot matmuls accumulating in PSUM, then apply
    h' = 0.9 * norm * (A @ g) + 0.1 * h0c
as a fused vector epilogue (tracking g = norm*h between steps).
"""
import sys

import numpy as np

sys.path.insert(0, "/opt/trn_rl_repo")

import concourse.bacc as bacc
import concourse.bass as bass
import concourse.mybir as mybir
import concourse.tile as tile
from concourse.bass_utils import run_bass_kernel_spmd

P = 128
NCORES = 8
C = 64
ALPHA = 0.1
A1, A2, A3 = 0.5, 0.3, 0.2
F32 = mybir.dt.float32
BF16 = mybir.dt.bfloat16
I16 = mybir.dt.int16
I32 = mybir.dt.int32


class Cfg:
    def __init__(self, n_nodes, n_edges, k_steps, tq=8, cb=4, use_collectives=True):
        self.N = n_nodes
        self.E = n_edges
        self.K = k_steps
        self.TQ = tq          # gather tiles per (block, quarter)
        self.CB = cb          # blocks per chunk
        self.B = None         # blocks per core (set by preprocess)
        self.use_collectives = use_collectives

    @property
    def SLOTS(self):
        return self.B * P

    @property
    def QROWS(self):
        return 2 * self.SLOTS


def preprocess(cfg, edge_src, edge_dst, norm):
    """Build the packed layout and per-core index/constant arrays."""
    N, TQ = cfg.N, cfg.TQ
    QCAP = TQ * P
    src = edge_src.astype(np.int64)
    dst = edge_dst.astype(np.int64)

    deg = np.bincount(dst, minlength=N)
    cum = np.cumsum(deg)
    # node -> core, balanced by in-degree
    cuts = [int(np.searchsorted(cum, (c + 1) * cfg.E / NCORES)) for c in range(NCORES - 1)]
    bounds = np.array([0] + [min(c + 1, N) for c in cuts] + [N])
    node_core = np.searchsorted(bounds[1:], np.arange(N), side="right").astype(np.int32)
    node_q = node_core // 2

    # per-node, per-source-quarter in-degree
    q_of_src = node_q[src]
    deg_q = np.bincount(dst * 4 + q_of_src, minlength=N * 4).reshape(N, 4)

    # bin-pack nodes of each core into blocks: <=128 nodes, <=QCAP edges/quarter.
    # First-fit-decreasing on max quarter degree packs blocks much closer to
    # the 128-node limit than in-order greedy (fewer blocks -> fewer gather
    # calls per step, which is the kernel's bottleneck).
    node_block = np.empty(N, np.int32)
    node_slot = np.empty(N, np.int32)
    blocks_per_core = []
    for c in range(NCORES):
        lo, hi = bounds[c], bounds[c + 1]
        nn = hi - lo
        if nn == 0:
            blocks_per_core.append(0)
            continue
        nodes = np.arange(lo, hi)
        order_n = nodes[np.argsort(-deg_q[lo:hi].max(axis=1), kind="stable")]
        tot = deg_q[lo:hi].sum(axis=0)
        nb = max(int(np.ceil(tot.max() / QCAP)), int(np.ceil(nn / P)))
        while True:
            qcs = np.zeros((nb, 4), np.int64)
            cnt = np.zeros(nb, np.int64)
            ok_all = True
            for n in order_n:
                d = deg_q[n]
                feas = (cnt < P) & np.all(qcs + d <= QCAP, axis=1)
                if not feas.any():
                    ok_all = False
                    break
                load = np.where(
                    feas[:, None], qcs + d, np.iinfo(np.int64).max
                ).max(axis=1)
                b = int(np.argmin(load))
                node_block[n] = b
                node_slot[n] = cnt[b]
                qcs[b] += d
                cnt[b] += 1
            if ok_all:
                break
            nb += 1
        blocks_per_core.append(nb)

    B = max(blocks_per_core)
    B = -(-B // cfg.CB) * cfg.CB  # pad to chunk multiple
    cfg.B = B
    SLOTS = cfg.SLOTS
    assert cfg.QROWS <= 32767, f"QROWS {cfg.QROWS} exceeds int16 range"

    pi_row = node_core.astype(np.int64) * SLOTS + node_block * P + node_slot

    # --- edge arrays ---
    c_e = node_core[dst]
    b_e = node_block[dst]
    sl_e = node_slot[dst]
    q_e = q_of_src
    gidx_e = (pi_row[src] - q_e.astype(np.int64) * cfg.QROWS).astype(np.int64)
    assert gidx_e.min() >= 0 and gidx_e.max() < cfg.QROWS

    run = ((c_e.astype(np.int64) * 4 + q_e) * B + b_e)
    order = np.lexsort((gidx_e, run))
    run_s = run[order]
    # rank within run
    starts = np.zeros(NCORES * 4 * B, np.int64)
    first = np.ones(len(run_s), bool)
    first[1:] = run_s[1:] != run_s[:-1]
    starts[run_s[first]] = np.flatnonzero(first)
    rank = np.arange(len(run_s)) - starts[run_s]
    assert rank.max() < QCAP, f"run overflow: {rank.max()} >= {QCAP}"

    QARR = B * QCAP  # per-(core, quarter) slot count
    IDX = np.zeros((NCORES, 4 * QARR), np.int16)
    SLOT = np.full((NCORES, 4 * QARR), 200.0, np.float32)
    pos = (q_e[order] * QARR + b_e[order].astype(np.int64) * QCAP + rank)
    IDX[c_e[order], pos] = gidx_e[order].astype(np.int16)
    SLOT[c_e[order], pos] = sl_e[order].astype(np.float32)

    # idxw [core, 128, (B+ceil(B/3))*QCAP/16]: partition group g in {0,1,2}
    # holds quarter g's indices for all blocks, then quarter 3's indices for
    # blocks with blk%3 == g (at column offset (B + blk//3)*QCAP/16).  Wrapped
    # (i%16, i//16) and replicated x2 within the group for the tx/rx pair.
    B3 = -(-B // 3)
    idxw = np.zeros((NCORES, P, (B + B3) * QCAP // 16), np.int16)
    wq = IDX.reshape(NCORES, 4, B, QCAP // 16, 16).transpose(0, 1, 2, 4, 3)
    for g in range(3):
        cols = wq[:, g].transpose(0, 2, 1, 3).reshape(NCORES, 16, B * QCAP // 16)
        idxw[:, 32 * g : 32 * g + 16, : B * QCAP // 16] = cols
        idxw[:, 32 * g + 16 : 32 * g + 32, : B * QCAP // 16] = cols
    for blk in range(B):
        g = blk % 3
        c0 = (B + blk // 3) * QCAP // 16
        w = wq[:, 3, blk]
        idxw[:, 32 * g : 32 * g + 16, c0 : c0 + QCAP // 16] = w
        idxw[:, 32 * g + 16 : 32 * g + 32, c0 : c0 + QCAP // 16] = w
    idxw = np.ascontiguousarray(idxw)
    # slot ids transposed: column j = global tile (q*B + b)*TQ + t
    slott = np.ascontiguousarray(
        SLOT.reshape(NCORES, 4 * B * TQ, P).transpose(0, 2, 1)
    )

    # --- per-slot constants, packed ---
    nrm = np.zeros((NCORES, SLOTS), np.float32)
    nrm[node_core, (node_block * P + node_slot)] = norm.reshape(-1).astype(np.float32)

    def pack_cols(v):  # [NCORES, SLOTS] -> [NCORES, 128, B]
        return np.ascontiguousarray(v.reshape(NCORES, B, P).transpose(0, 2, 1))

    packs = {
        "nfull": pack_cols(nrm),
        "n01": pack_cols(0.1 * nrm),
        "avec": pack_cols(0.9 * nrm * nrm),
        "avlast": pack_cols(0.9 * nrm),
    }

    return {
        "pi_row": pi_row,
        "node_core": node_core,
        "node_pos": node_block * P + node_slot,
        "idxw": idxw,
        "slott": slott,
        "packs": packs,
    }


def build(cfg):
    """Build the SPMD Bass program (same graph on all 8 cores)."""
    B, TQ, CB, K = cfg.B, cfg.TQ, cfg.CB, cfg.K
    SLOTS, QROWS = cfg.SLOTS, cfg.QROWS
    QCAP = TQ * P
    NCHUNK = B // CB
    B3 = -(-B // 3)
    IDXCOLS = (B + B3) * QCAP // 16

    nc = bacc.Bacc("TRN2", target_bir_lowering=False, debug=False, num_devices=NCORES, num_swdge_queues=3)

    # inputs
    x1t = nc.declare_dram_parameter("x1t", [512, SLOTS], BF16, isOutput=False)
    x2t = nc.declare_dram_parameter("x2t", [256, SLOTS], BF16, isOutput=False)
    x3t = nc.declare_dram_parameter("x3t", [128, SLOTS], BF16, isOutput=False)
    w1 = nc.declare_dram_parameter("w1", [512, 256], BF16, isOutput=False)
    w2 = nc.declare_dram_parameter("w2", [256, 256], BF16, isOutput=False)
    w3 = nc.declare_dram_parameter("w3", [128, 256], BF16, isOutput=False)
    b1 = nc.declare_dram_parameter("b1", [P, 2], F32, isOutput=False)
    b2 = nc.declare_dram_parameter("b2", [P, 2], F32, isOutput=False)
    b3 = nc.declare_dram_parameter("b3", [P, 2], F32, isOutput=False)
    w2p1 = nc.declare_dram_parameter("w2p1", [256, C], BF16, isOutput=False)
    w2p2 = nc.declare_dram_parameter("w2p2", [256, C], BF16, isOutput=False)
    w2p3 = nc.declare_dram_parameter("w2p3", [256, C], BF16, isOutput=False)
    beta = nc.declare_dram_parameter("beta", [P, C], F32, isOutput=False)
    idxw = nc.declare_dram_parameter("idxw", [P, IDXCOLS], I16, isOutput=False)
    slott = nc.declare_dram_parameter("slott", [P, 4 * B * TQ], F32, isOutput=False)
    nfull = nc.declare_dram_parameter("nfull", [P, B], F32, isOutput=False)
    n01 = nc.declare_dram_parameter("n01", [P, B], F32, isOutput=False)
    avec = nc.declare_dram_parameter("avec", [P, B], F32, isOutput=False)
    avlast = nc.declare_dram_parameter("avlast", [P, B], F32, isOutput=False)

    out = nc.declare_dram_parameter("out", [SLOTS, C], F32, isOutput=True)

    # internal DRAM
    shard = nc.dram_tensor("shard", [SLOTS, C], F32)
    tabA = nc.dram_tensor("tabA", [NCORES * SLOTS, C], F32, addr_space="Shared")
    tabB = nc.dram_tensor("tabB", [NCORES * SLOTS, C], F32, addr_space="Shared")

    rg = [list(range(NCORES))]
    WDEFS = [(x1t, 4, w1, b1, w2p1), (x2t, 2, w2, b2, w2p2), (x3t, 1, w3, b3, w2p3)]

    with tile.TileContext(nc) as tc:
        with (
            tc.tile_pool(name="const", bufs=1) as cpool,
            tc.tile_pool(name="work", bufs=3) as wp,
        ):
            # ---------- resident constants ----------
            idx_sb = cpool.tile([P, IDXCOLS], I16)
            nc.sync.dma_start(out=idx_sb[:], in_=idxw[:])
            slott_sb = cpool.tile([P, 4 * B * TQ], F32)
            nc.sync.dma_start(out=slott_sb[:], in_=slott[:])
            nfull_sb = cpool.tile([P, B], F32)
            nc.sync.dma_start(out=nfull_sb[:], in_=nfull[:])
            n01_sb = cpool.tile([P, B], F32)
            nc.sync.dma_start(out=n01_sb[:], in_=n01[:])
            avec_sb = cpool.tile([P, B], F32)
            nc.sync.dma_start(out=avec_sb[:], in_=avec[:])
            avlast_sb = cpool.tile([P, B], F32)
            nc.sync.dma_start(out=avlast_sb[:], in_=avlast[:])
            beta_sb = cpool.tile([P, C], F32)
            nc.sync.dma_start(out=beta_sb[:], in_=beta[:])
            h0c_sb = cpool.tile([P, B * C], F32)

            # ---------- MLP phase ----------
            with (
                tc.tile_pool(name="mlpw", bufs=1) as mw,
                tc.tile_pool(name="psmlp", bufs=2, space="PSUM") as pmlp,
            ):
                w_sb = []
                for wi, (xt, nkt, w, b, w2x) in enumerate(WDEFS):
                    wt = mw.tile([P, nkt, 256], BF16, tag=f"wt{wi}")
                    nc.sync.dma_start(
                        out=wt[:], in_=w[:].rearrange("(kt p) h -> p kt h", p=P)
                    )
                    bt = mw.tile([P, 2], F32, tag=f"bt{wi}")
                    nc.sync.dma_start(out=bt[:], in_=b[:])
                    w2t = mw.tile([P, 2, C], BF16, tag=f"w2t{wi}")
                    nc.sync.dma_start(
                        out=w2t[:], in_=w2x[:].rearrange("(ht p) o -> p ht o", p=P)
                    )
                    w_sb.append((wt, bt, w2t))

                for blk in range(B):
                    x_sb = []
                    for bi2, ((xt, nkt, _, _, _), _w) in enumerate(zip(WDEFS, w_sb)):
                        xtile = wp.tile([P, nkt, P], BF16, tag=f"xin{bi2}")
                        nc.sync.dma_start(
                            out=xtile[:],
                            in_=xt[:].rearrange("(kt p) n -> p kt n", p=P)[
                                :, :, blk * P : (blk + 1) * P
                            ],
                        )
                        x_sb.append(xtile)
                    ps2 = pmlp.tile([P, C], F32, tag="ps2")
                    first = True
                    for bi, ((xt, nkt, _, _, _), (wt, bt, w2t)) in enumerate(
                        zip(WDEFS, w_sb)
                    ):
                        for half in range(2):
                            ps1 = pmlp.tile([P, P], F32, tag="ps1")
                            for kt in range(nkt):
                                nc.tensor.matmul(
                                    out=ps1[:],
                                    lhsT=wt[:, kt, half * P : (half + 1) * P],
                                    rhs=x_sb[bi][:, kt, :],
                                    start=(kt == 0),
                                    stop=(kt == nkt - 1),
                                )
                            r = wp.tile([P, P], BF16, tag="relu")
                            nc.scalar.activation(
                                out=r[:],
                                in_=ps1[:],
                                func=mybir.ActivationFunctionType.Relu,
                                bias=bt[:, half : half + 1],
                            )
                            nc.tensor.matmul(
                                out=ps2[:],
                                lhsT=r[:],
                                rhs=w2t[:, half, :],
                                start=first,
                                stop=(bi == 2 and half == 1),
                            )
                            first = False
                    # h0c = ps2 + beta ; g0 = norm * h0c
                    h0col = h0c_sb[:, blk * C : (blk + 1) * C]
                    nc.vector.tensor_add(out=h0col, in0=ps2[:], in1=beta_sb[:])
                    g0 = wp.tile([P, C], F32, tag="g0")
                    nc.vector.tensor_scalar_mul(
                        g0[:], h0col, nfull_sb[:, blk : blk + 1]
                    )
                    nc.sync.dma_start(
                        out=shard[blk * P : (blk + 1) * P, :], in_=g0[:]
                    )

            if cfg.use_collectives:
                nc.gpsimd.collective_compute(
                    "AllGather",
                    mybir.AluOpType.bypass,
                    replica_groups=rg,
                    ins=[shard[:]],
                    outs=[tabA[:]],
                )
            else:
                for r in range(NCORES):
                    nc.sync.dma_start(
                        out=tabA[r * SLOTS : (r + 1) * SLOTS, :], in_=shard[:]
                    )

            # ---------- propagation ----------
            iota3 = cpool.tile([P, 4 * TQ * P], F32)
            nc.gpsimd.iota(
                iota3[:],
                pattern=[[0, 4 * TQ], [1, P]],
                base=0,
                channel_multiplier=0,
                allow_small_or_imprecise_dtypes=True,
            )

            with (
                tc.tile_pool(name="msg", bufs=5) as mp,
                tc.tile_pool(name="msgb", bufs=5) as mbp,
                tc.tile_pool(name="sel", bufs=4) as sp,
                tc.tile_pool(name="gout", bufs=2) as gp,
                tc.tile_pool(name="psprop", bufs=8, space="PSUM") as ppp,
            ):
                for step in range(1, K + 1):
                    src_tab = tabA if step % 2 == 1 else tabB
                    dst_tab = tabB if step % 2 == 1 else tabA
                    last = step == K
                    for chunk in range(NCHUNK):
                        gout = gp.tile([P, CB, C], F32, tag="go")
                        for j in range(CB):
                            blk = chunk * CB + j
                            msgb = []
                            for q in range(4):
                                m = mp.tile([P, TQ, C], F32, tag=f"m{q}")
                                if q < 3:
                                    qn = q
                                    col0 = blk * QCAP // 16
                                else:
                                    qn = blk % 3
                                    col0 = (B + blk // 3) * QCAP // 16
                                nc.gpsimd.dma_gather(
                                    m[:],
                                    src_tab[q * QROWS : (q + 1) * QROWS, :],
                                    idx_sb[:, col0 : col0 + QCAP // 16],
                                    QCAP,
                                    QCAP,
                                    C,
                                    queue_num=qn,
                                )
                                mb = mbp.tile([P, TQ, C], BF16, tag=f"mb{q}")
                                nc.scalar.activation(
                                    out=mb[:],
                                    in_=m[:],
                                    func=mybir.ActivationFunctionType.Copy,
                                )
                                msgb.append(mb)
                            s4 = sp.tile([P, 4, TQ, P], BF16, tag="s4")
                            nc.vector.tensor_tensor(
                                out=s4[:],
                                in0=slott_sb[:]
                                .rearrange("p (q b t) -> p q b t", q=4, b=B)[
                                    :, :, blk, :
                                ]
                                .to_broadcast([P, 4, TQ, P]),
                                in1=iota3[:].rearrange(
                                    "p (q t d) -> p q t d", q=4, t=TQ
                                ),
                                op=mybir.AluOpType.is_equal,
                            )
                            ps = ppp.tile([P, C], F32, tag="prop")
                            for q in range(4):
                                for t in range(TQ):
                                    nc.tensor.matmul(
                                        out=ps[:],
                                        lhsT=s4[:, q, t, :],
                                        rhs=msgb[q][:, t, :],
                                        start=(q == 0 and t == 0),
                                        stop=(q == 3 and t == TQ - 1),
                                    )
                            # epilogue: g' = avec*ps + n01*h0c   (or h for last step)
                            tmp = wp.tile([P, C], F32, tag="tmp")
                            h0col = h0c_sb[:, blk * C : (blk + 1) * C]
                            if last:
                                nc.vector.tensor_scalar_mul(tmp[:], h0col, ALPHA)
                                av = avlast_sb
                            else:
                                nc.vector.tensor_scalar_mul(
                                    tmp[:], h0col, n01_sb[:, blk : blk + 1]
                                )
                                av = avec_sb
                            nc.vector.scalar_tensor_tensor(
                                out=gout[:, j, :],
                                in0=ps[:],
                                scalar=av[:, blk : blk + 1],
                                in1=tmp[:],
                                op0=mybir.AluOpType.mult,
                                op1=mybir.AluOpType.add,
                            )
                        dst_rows = slice(chunk * CB * P, (chunk + 1) * CB * P)
                        dst_t = out if last else shard
                        nc.sync.dma_start(
                            out=dst_t[dst_rows, :].rearrange(
                                "(cb p) c -> p cb c", p=P
                            ),
                            in_=gout[:],
                        )
                    if not last:
                        if cfg.use_collectives:
                            nc.gpsimd.collective_compute(
                                "AllGather",
                                mybir.AluOpType.bypass,
                                replica_groups=rg,
                                ins=[shard[:]],
                                outs=[dst_tab[:]],
                            )
                        else:
                            for r in range(NCORES):
                                nc.sync.dma_start(
                                    out=dst_tab[r * SLOTS : (r + 1) * SLOTS, :],
                                    in_=shard[:],
                                )

    nc.compile()
    return nc


def make_in_maps(cfg, pre, inputs):
    """Per-core input dicts."""
    f1 = np.asarray(inputs["features1"], np.float32)
    f2 = np.asarray(inputs["features2"], np.float32)
    f3 = np.asarray(inputs["features3"], np.float32)
    SLOTS = cfg.SLOTS
    node_core, node_pos = pre["node_core"], pre["node_pos"]

    import ml_dtypes

    BF = ml_dtypes.bfloat16

    def pack_T(feat):
        D = feat.shape[1]
        o = np.zeros((NCORES, D, SLOTS), BF)
        o[node_core, :, node_pos] = feat.astype(BF)
        return o

    X1, X2, X3 = pack_T(f1), pack_T(f2), pack_T(f3)

    def bias_fold(b):  # [256] -> [128, 2]
        return np.ascontiguousarray(np.asarray(b, np.float32).reshape(2, P).T)

    beta = (
        A1 * np.asarray(inputs["b1_1"])
        + A2 * np.asarray(inputs["b2_1"])
        + A3 * np.asarray(inputs["b3_1"])
    ).astype(np.float32)
    beta_rep = np.ascontiguousarray(np.broadcast_to(beta, (P, C)))

    common = {
        "w1": np.asarray(inputs["w1_0"], np.float32).astype(BF),
        "w2": np.asarray(inputs["w2_0"], np.float32).astype(BF),
        "w3": np.asarray(inputs["w3_0"], np.float32).astype(BF),
        "b1": bias_fold(inputs["b1_0"]),
        "b2": bias_fold(inputs["b2_0"]),
        "b3": bias_fold(inputs["b3_0"]),
        "w2p1": (A1 * np.asarray(inputs["w1_1"], np.float32)).astype(BF),
        "w2p2": (A2 * np.asarray(inputs["w2_1"], np.float32)).astype(BF),
        "w2p3": (A3 * np.asarray(inputs["w3_1"], np.float32)).astype(BF),
        "beta": beta_rep,
    }
    in_maps = []
    for c in range(NCORES):
        m = dict(common)
        m["x1t"] = np.ascontiguousarray(X1[c])
        m["x2t"] = np.ascontiguousarray(X2[c])
        m["x3t"] = np.ascontiguousarray(X3[c])
        m["idxw"] = pre["idxw"][c]
        m["slott"] = pre["slott"][c]
        for k in ("nfull", "n01", "avec", "avlast"):
            m[k] = pre["packs"][k][c]
        in_maps.append(m)
    return in_maps


_CACHE = {}


def run(inputs, cfg, **spmd_kwargs):
    pre_key = "pre"
    if pre_key not in _CACHE:
        _CACHE[pre_key] = preprocess(
            cfg,
            np.asarray(inputs["edge_src"]),
            np.asarray(inputs["edge_dst"]),
            np.asarray(inputs["norm"]),
        )
        _CACHE["nc"] = build(cfg)
    pre = _CACHE[pre_key]
    nc = _CACHE["nc"]
    in_maps = make_in_maps(cfg, pre, inputs)
    res = run_bass_kernel_spmd(nc, in_maps, core_ids=list(range(NCORES)), **spmd_kwargs)
    full = np.concatenate([res.results[c]["out"] for c in range(NCORES)], axis=0)
    out = full[pre["pi_row"]]
    return out.astype(np.float32), res


def kernel(**inputs):
    cfg = Cfg(n_nodes=100000, n_edges=3200000, k_steps=10)
    out, _ = run(inputs, cfg)
    return out


# revision 18
# speedup vs baseline: 1.0827x; 1.0827x over previous
"""APPNP (3-branch MLP + fused linear propagation) on 8 Trainium2 NeuronCores.

Math: the APPNP propagation is linear in h, so
    a1*P(h1) + a2*P(h2) + a3*P(h3) = P(a1*h1 + a2*h2 + a3*h3).
We compute h0c = sum_i a_i * mlp_i(x_i) once (on device), then run K=10
propagation steps on the single fused [N, 64] tensor.

Distribution: nodes are sharded across 8 cores (balanced by in-degree) and
bin-packed into blocks of 128 "slots" (a permuted, padded node order).  Each
step: AllGather the pre-scaled node states g = norm*h into a full table in
each core's HBM, dma_gather the per-edge source rows (edges grouped by
destination block and source quarter so int16 gather indices stay in range),
segment-sum via one-hot matmuls accumulating in PSUM, then apply
    h' = 0.9 * norm * (A @ g) + 0.1 * h0c
as a fused vector epilogue (tracking g = norm*h between steps).
"""
import sys

import numpy as np

sys.path.insert(0, "/opt/trn_rl_repo")

import concourse.bacc as bacc
import concourse.bass as bass
import concourse.mybir as mybir
import concourse.tile as tile
from concourse.bass_utils import run_bass_kernel_spmd

P = 128
NCORES = 8
C = 64
ALPHA = 0.1
A1, A2, A3 = 0.5, 0.3, 0.2
F32 = mybir.dt.float32
BF16 = mybir.dt.bfloat16
I16 = mybir.dt.int16
I32 = mybir.dt.int32


class Cfg:
    def __init__(self, n_nodes, n_edges, k_steps, tq=8, cb=4, use_collectives=True):
        self.N = n_nodes
        self.E = n_edges
        self.K = k_steps
        self.TQ = tq          # gather tiles per (block, quarter)
        self.CB = cb          # blocks per chunk
        self.B = None         # blocks per core (set by preprocess)
        self.use_collectives = use_collectives

    @property
    def SLOTS(self):
        return self.B * P

    @property
    def QROWS(self):
        return 2 * self.SLOTS


def preprocess(cfg, edge_src, edge_dst, norm):
    """Build the packed layout and per-core index/constant arrays."""
    N, TQ = cfg.N, cfg.TQ
    QCAP = TQ * P
    src = edge_src.astype(np.int64)
    dst = edge_dst.astype(np.int64)

    deg = np.bincount(dst, minlength=N)
    cum = np.cumsum(deg)
    # node -> core, balanced by in-degree
    cuts = [int(np.searchsorted(cum, (c + 1) * cfg.E / NCORES)) for c in range(NCORES - 1)]
    bounds = np.array([0] + [min(c + 1, N) for c in cuts] + [N])
    node_core = np.searchsorted(bounds[1:], np.arange(N), side="right").astype(np.int32)
    # gather-region group of a node: round-robin within its core.  Region j of
    # the table is filled by every core's block-group j, so a step's sub-
    # AllGather j can launch as soon as block-group j's epilogues finish.
    node_q = np.empty(N, np.int32)
    for c in range(NCORES):
        lo, hi = bounds[c], bounds[c + 1]
        node_q[lo:hi] = np.arange(hi - lo) % 4

    # per-node, per-source-group in-degree
    q_of_src = node_q[src]
    deg_q = np.bincount(dst * 4 + q_of_src, minlength=N * 4).reshape(N, 4)

    # bin-pack nodes of each core into blocks: <=128 nodes, <=QCAP edges/quarter.
    # First-fit-decreasing on max quarter degree packs blocks much closer to
    # the 128-node limit than in-order greedy (fewer blocks -> fewer gather
    # calls per step, which is the kernel's bottleneck).
    node_bw = np.empty(N, np.int32)
    node_slot = np.empty(N, np.int32)
    blocks_per_group = np.zeros((NCORES, 4), np.int64)
    for c in range(NCORES):
        lo, hi = bounds[c], bounds[c + 1]
        for qg in range(4):
            sel = lo + np.flatnonzero(node_q[lo:hi] == qg)
            nn = len(sel)
            if nn == 0:
                continue
            order_n = sel[np.argsort(-deg_q[sel].max(axis=1), kind="stable")]
            tot = deg_q[sel].sum(axis=0)
            nb = max(int(np.ceil(tot.max() / QCAP)), int(np.ceil(nn / P)))
            while True:
                qcs = np.zeros((nb, 4), np.int64)
                cnt = np.zeros(nb, np.int64)
                ok_all = True
                for n in order_n:
                    d = deg_q[n]
                    feas = (cnt < P) & np.all(qcs + d <= QCAP, axis=1)
                    if not feas.any():
                        ok_all = False
                        break
                    load = np.where(
                        feas[:, None], qcs + d, np.iinfo(np.int64).max
                    ).max(axis=1)
                    b = int(np.argmin(load))
                    node_bw[n] = b
                    node_slot[n] = cnt[b]
                    qcs[b] += d
                    cnt[b] += 1
                if ok_all:
                    break
                nb += 1
            blocks_per_group[c, qg] = nb

    B4 = int(blocks_per_group.max())
    B = 4 * B4
    cfg.B = B
    cfg.B4 = B4
    node_block = node_q * B4 + node_bw
    SLOTS = cfg.SLOTS
    assert cfg.QROWS <= 32767, f"QROWS {cfg.QROWS} exceeds int16 range"

    # table row: region-major [group j][core][block-within-group][slot]
    pi_row = (
        node_q.astype(np.int64) * cfg.QROWS
        + node_core.astype(np.int64) * (B4 * P)
        + node_bw.astype(np.int64) * P
        + node_slot
    )

    # --- edge arrays ---
    c_e = node_core[dst]
    b_e = node_block[dst]
    sl_e = node_slot[dst]
    q_e = q_of_src
    gidx_e = (pi_row[src] - q_e.astype(np.int64) * cfg.QROWS).astype(np.int64)
    assert gidx_e.min() >= 0 and gidx_e.max() < cfg.QROWS

    run = ((c_e.astype(np.int64) * 4 + q_e) * B + b_e)
    order = np.lexsort((gidx_e, run))
    run_s = run[order]
    # rank within run
    starts = np.zeros(NCORES * 4 * B, np.int64)
    first = np.ones(len(run_s), bool)
    first[1:] = run_s[1:] != run_s[:-1]
    starts[run_s[first]] = np.flatnonzero(first)
    rank = np.arange(len(run_s)) - starts[run_s]
    assert rank.max() < QCAP, f"run overflow: {rank.max()} >= {QCAP}"

    QARR = B * QCAP  # per-(core, quarter) slot count
    IDX = np.zeros((NCORES, 4 * QARR), np.int16)
    SLOT = np.full((NCORES, 4 * QARR), 200.0, np.float32)
    pos = (q_e[order] * QARR + b_e[order].astype(np.int64) * QCAP + rank)
    IDX[c_e[order], pos] = gidx_e[order].astype(np.int16)
    SLOT[c_e[order], pos] = sl_e[order].astype(np.float32)

    # idxw [core, 128, (B+ceil(B/3))*QCAP/16]: partition group g in {0,1,2}
    # holds quarter g's indices for all blocks, then quarter 3's indices for
    # blocks with blk%3 == g (at column offset (B + blk//3)*QCAP/16).  Wrapped
    # (i%16, i//16) and replicated x2 within the group for the tx/rx pair.
    B3 = -(-B // 3)
    idxw = np.zeros((NCORES, P, (B + B3) * QCAP // 16), np.int16)
    wq = IDX.reshape(NCORES, 4, B, QCAP // 16, 16).transpose(0, 1, 2, 4, 3)
    for g in range(3):
        cols = wq[:, g].transpose(0, 2, 1, 3).reshape(NCORES, 16, B * QCAP // 16)
        idxw[:, 32 * g : 32 * g + 16, : B * QCAP // 16] = cols
        idxw[:, 32 * g + 16 : 32 * g + 32, : B * QCAP // 16] = cols
    for blk in range(B):
        g = blk % 3
        c0 = (B + blk // 3) * QCAP // 16
        w = wq[:, 3, blk]
        idxw[:, 32 * g : 32 * g + 16, c0 : c0 + QCAP // 16] = w
        idxw[:, 32 * g + 16 : 32 * g + 32, c0 : c0 + QCAP // 16] = w
    idxw = np.ascontiguousarray(idxw)
    # slot ids transposed: column j = global tile (q*B + b)*TQ + t
    slott = np.ascontiguousarray(
        SLOT.reshape(NCORES, 4 * B * TQ, P).transpose(0, 2, 1)
    )

    # --- per-slot constants, packed ---
    nrm = np.zeros((NCORES, SLOTS), np.float32)
    nrm[node_core, (node_block * P + node_slot)] = norm.reshape(-1).astype(np.float32)

    def pack_cols(v):  # [NCORES, SLOTS] -> [NCORES, 128, B]
        return np.ascontiguousarray(v.reshape(NCORES, B, P).transpose(0, 2, 1))

    packs = {
        "nfull": pack_cols(nrm),
        "n01": pack_cols(0.1 * nrm),
        "avec": pack_cols(0.9 * nrm * nrm),
        "avlast": pack_cols(0.9 * nrm),
    }

    return {
        "pi_row": pi_row,
        "out_row": node_core.astype(np.int64) * SLOTS + node_block * P + node_slot,
        "node_core": node_core,
        "node_pos": node_block * P + node_slot,
        "idxw": idxw,
        "slott": slott,
        "packs": packs,
    }


def build(cfg):
    """Build the SPMD Bass program (same graph on all 8 cores)."""
    B, TQ, CB, K = cfg.B, cfg.TQ, cfg.CB, cfg.K
    B4 = B // 4
    SLOTS, QROWS = cfg.SLOTS, cfg.QROWS
    QCAP = TQ * P
    NCHUNK = B // CB
    B3 = -(-B // 3)
    IDXCOLS = (B + B3) * QCAP // 16

    nc = bacc.Bacc("TRN2", target_bir_lowering=False, debug=False, num_devices=NCORES, num_swdge_queues=3)

    # inputs
    x1t = nc.declare_dram_parameter("x1t", [512, SLOTS], BF16, isOutput=False)
    x2t = nc.declare_dram_parameter("x2t", [256, SLOTS], BF16, isOutput=False)
    x3t = nc.declare_dram_parameter("x3t", [128, SLOTS], BF16, isOutput=False)
    w1 = nc.declare_dram_parameter("w1", [512, 256], BF16, isOutput=False)
    w2 = nc.declare_dram_parameter("w2", [256, 256], BF16, isOutput=False)
    w3 = nc.declare_dram_parameter("w3", [128, 256], BF16, isOutput=False)
    b1 = nc.declare_dram_parameter("b1", [P, 2], F32, isOutput=False)
    b2 = nc.declare_dram_parameter("b2", [P, 2], F32, isOutput=False)
    b3 = nc.declare_dram_parameter("b3", [P, 2], F32, isOutput=False)
    w2p1 = nc.declare_dram_parameter("w2p1", [256, C], BF16, isOutput=False)
    w2p2 = nc.declare_dram_parameter("w2p2", [256, C], BF16, isOutput=False)
    w2p3 = nc.declare_dram_parameter("w2p3", [256, C], BF16, isOutput=False)
    beta = nc.declare_dram_parameter("beta", [P, C], F32, isOutput=False)
    idxw = nc.declare_dram_parameter("idxw", [P, IDXCOLS], I16, isOutput=False)
    slott = nc.declare_dram_parameter("slott", [P, 4 * B * TQ], F32, isOutput=False)
    nfull = nc.declare_dram_parameter("nfull", [P, B], F32, isOutput=False)
    n01 = nc.declare_dram_parameter("n01", [P, B], F32, isOutput=False)
    avec = nc.declare_dram_parameter("avec", [P, B], F32, isOutput=False)
    avlast = nc.declare_dram_parameter("avlast", [P, B], F32, isOutput=False)

    out = nc.declare_dram_parameter("out", [SLOTS, C], F32, isOutput=True)

    # internal DRAM
    shard = nc.dram_tensor("shard", [SLOTS, C], F32)
    tabA = nc.dram_tensor("tabA", [NCORES * SLOTS, C], F32, addr_space="Shared")
    tabB = nc.dram_tensor("tabB", [NCORES * SLOTS, C], F32, addr_space="Shared")

    rg = [list(range(NCORES))]
    GROW = B4 * P  # rows per block-group in the shard

    def emit_sub_ag(nc_, dst_tab_, j):
        ins_ap = shard[j * GROW : (j + 1) * GROW, :]
        outs_ap = dst_tab_[j * QROWS : (j + 1) * QROWS, :]
        if cfg.use_collectives:
            nc_.gpsimd.collective_compute(
                "AllGather",
                mybir.AluOpType.bypass,
                replica_groups=rg,
                ins=[ins_ap],
                outs=[outs_ap],
            )
        else:
            for r in range(NCORES):
                nc_.sync.dma_start(
                    out=dst_tab_[
                        j * QROWS + r * GROW : j * QROWS + (r + 1) * GROW, :
                    ],
                    in_=ins_ap,
                )
    WDEFS = [(x1t, 4, w1, b1, w2p1), (x2t, 2, w2, b2, w2p2), (x3t, 1, w3, b3, w2p3)]

    with tile.TileContext(nc) as tc:
        with (
            tc.tile_pool(name="const", bufs=1) as cpool,
            tc.tile_pool(name="work", bufs=3) as wp,
        ):
            # ---------- resident constants ----------
            idx_sb = cpool.tile([P, IDXCOLS], I16)
            nc.sync.dma_start(out=idx_sb[:], in_=idxw[:])
            slott_sb = cpool.tile([P, 4 * B * TQ], F32)
            nc.sync.dma_start(out=slott_sb[:], in_=slott[:])
            nfull_sb = cpool.tile([P, B], F32)
            nc.sync.dma_start(out=nfull_sb[:], in_=nfull[:])
            n01_sb = cpool.tile([P, B], F32)
            nc.sync.dma_start(out=n01_sb[:], in_=n01[:])
            avec_sb = cpool.tile([P, B], F32)
            nc.sync.dma_start(out=avec_sb[:], in_=avec[:])
            avlast_sb = cpool.tile([P, B], F32)
            nc.sync.dma_start(out=avlast_sb[:], in_=avlast[:])
            beta_sb = cpool.tile([P, C], F32)
            nc.sync.dma_start(out=beta_sb[:], in_=beta[:])
            h0c_sb = cpool.tile([P, B * C], F32)

            # ---------- MLP phase ----------
            with (
                tc.tile_pool(name="mlpw", bufs=1) as mw,
                tc.tile_pool(name="psmlp", bufs=2, space="PSUM") as pmlp,
            ):
                w_sb = []
                for wi, (xt, nkt, w, b, w2x) in enumerate(WDEFS):
                    wt = mw.tile([P, nkt, 256], BF16, tag=f"wt{wi}")
                    nc.sync.dma_start(
                        out=wt[:], in_=w[:].rearrange("(kt p) h -> p kt h", p=P)
                    )
                    bt = mw.tile([P, 2], F32, tag=f"bt{wi}")
                    nc.sync.dma_start(out=bt[:], in_=b[:])
                    w2t = mw.tile([P, 2, C], BF16, tag=f"w2t{wi}")
                    nc.sync.dma_start(
                        out=w2t[:], in_=w2x[:].rearrange("(ht p) o -> p ht o", p=P)
                    )
                    w_sb.append((wt, bt, w2t))

                for blk in range(B):
                    x_sb = []
                    for bi2, ((xt, nkt, _, _, _), _w) in enumerate(zip(WDEFS, w_sb)):
                        xtile = wp.tile([P, nkt, P], BF16, tag=f"xin{bi2}")
                        nc.sync.dma_start(
                            out=xtile[:],
                            in_=xt[:].rearrange("(kt p) n -> p kt n", p=P)[
                                :, :, blk * P : (blk + 1) * P
                            ],
                        )
                        x_sb.append(xtile)
                    ps2 = pmlp.tile([P, C], F32, tag="ps2")
                    first = True
                    for bi, ((xt, nkt, _, _, _), (wt, bt, w2t)) in enumerate(
                        zip(WDEFS, w_sb)
                    ):
                        for half in range(2):
                            ps1 = pmlp.tile([P, P], F32, tag="ps1")
                            for kt in range(nkt):
                                nc.tensor.matmul(
                                    out=ps1[:],
                                    lhsT=wt[:, kt, half * P : (half + 1) * P],
                                    rhs=x_sb[bi][:, kt, :],
                                    start=(kt == 0),
                                    stop=(kt == nkt - 1),
                                )
                            r = wp.tile([P, P], BF16, tag="relu")
                            nc.scalar.activation(
                                out=r[:],
                                in_=ps1[:],
                                func=mybir.ActivationFunctionType.Relu,
                                bias=bt[:, half : half + 1],
                            )
                            nc.tensor.matmul(
                                out=ps2[:],
                                lhsT=r[:],
                                rhs=w2t[:, half, :],
                                start=first,
                                stop=(bi == 2 and half == 1),
                            )
                            first = False
                    # h0c = ps2 + beta ; g0 = norm * h0c
                    h0col = h0c_sb[:, blk * C : (blk + 1) * C]
                    nc.vector.tensor_add(out=h0col, in0=ps2[:], in1=beta_sb[:])
                    g0 = wp.tile([P, C], F32, tag="g0")
                    nc.vector.tensor_scalar_mul(
                        g0[:], h0col, nfull_sb[:, blk : blk + 1]
                    )
                    nc.sync.dma_start(
                        out=shard[blk * P : (blk + 1) * P, :], in_=g0[:]
                    )
                    if (blk + 1) % B4 == 0:
                        emit_sub_ag(nc, tabA, blk // B4)


            # ---------- propagation ----------
            iota3 = cpool.tile([P, 4 * TQ * P], F32)
            nc.gpsimd.iota(
                iota3[:],
                pattern=[[0, 4 * TQ], [1, P]],
                base=0,
                channel_multiplier=0,
                allow_small_or_imprecise_dtypes=True,
            )

            with (
                tc.tile_pool(name="msg", bufs=5) as mp,
                tc.tile_pool(name="msgb", bufs=5) as mbp,
                tc.tile_pool(name="sel", bufs=4) as sp,
                tc.tile_pool(name="gout", bufs=2) as gp,
                tc.tile_pool(name="psprop", bufs=8, space="PSUM") as ppp,
            ):
                for step in range(1, K + 1):
                    src_tab = tabA if step % 2 == 1 else tabB
                    dst_tab = tabB if step % 2 == 1 else tabA
                    last = step == K
                    for chunk in range(NCHUNK):
                        gout = gp.tile([P, CB, C], F32, tag="go")
                        for j in range(CB):
                            blk = chunk * CB + j
                            msgb = []
                            for q in range(4):
                                m = mp.tile([P, TQ, C], F32, tag=f"m{q}")
                                if q < 3:
                                    qn = q
                                    col0 = blk * QCAP // 16
                                else:
                                    qn = blk % 3
                                    col0 = (B + blk // 3) * QCAP // 16
                                nc.gpsimd.dma_gather(
                                    m[:],
                                    src_tab[q * QROWS : (q + 1) * QROWS, :],
                                    idx_sb[:, col0 : col0 + QCAP // 16],
                                    QCAP,
                                    QCAP,
                                    C,
                                    queue_num=qn,
                                )
                                mb = mbp.tile([P, TQ, C], BF16, tag=f"mb{q}")
                                nc.scalar.activation(
                                    out=mb[:],
                                    in_=m[:],
                                    func=mybir.ActivationFunctionType.Copy,
                                )
                                msgb.append(mb)
                            s4 = sp.tile([P, 4, TQ, P], BF16, tag="s4")
                            nc.vector.tensor_tensor(
                                out=s4[:],
                                in0=slott_sb[:]
                                .rearrange("p (q b t) -> p q b t", q=4, b=B)[
                                    :, :, blk, :
                                ]
                                .to_broadcast([P, 4, TQ, P]),
                                in1=iota3[:].rearrange(
                                    "p (q t d) -> p q t d", q=4, t=TQ
                                ),
                                op=mybir.AluOpType.is_equal,
                            )
                            ps = ppp.tile([P, C], F32, tag="prop")
                            for q in range(4):
                                for t in range(TQ):
                                    nc.tensor.matmul(
                                        out=ps[:],
                                        lhsT=s4[:, q, t, :],
                                        rhs=msgb[q][:, t, :],
                                        start=(q == 0 and t == 0),
                                        stop=(q == 3 and t == TQ - 1),
                                    )
                            # epilogue: g' = avec*ps + n01*h0c   (or h for last step)
                            tmp = wp.tile([P, C], F32, tag="tmp")
                            h0col = h0c_sb[:, blk * C : (blk + 1) * C]
                            if last:
                                nc.vector.tensor_scalar_mul(tmp[:], h0col, ALPHA)
                                av = avlast_sb
                            else:
                                nc.vector.tensor_scalar_mul(
                                    tmp[:], h0col, n01_sb[:, blk : blk + 1]
                                )
                                av = avec_sb
                            nc.vector.scalar_tensor_tensor(
                                out=gout[:, j, :],
                                in0=ps[:],
                                scalar=av[:, blk : blk + 1],
                                in1=tmp[:],
                                op0=mybir.AluOpType.mult,
                                op1=mybir.AluOpType.add,
                            )
                        dst_rows = slice(chunk * CB * P, (chunk + 1) * CB * P)
                        dst_t = out if last else shard
                        nc.sync.dma_start(
                            out=dst_t[dst_rows, :].rearrange(
                                "(cb p) c -> p cb c", p=P
                            ),
                            in_=gout[:],
                        )
                        if not last:
                            for j in range(4):
                                if -(-(j + 1) * B4 // CB) - 1 == chunk:
                                    emit_sub_ag(nc, dst_tab, j)

    nc.compile()
    return nc


def make_in_maps(cfg, pre, inputs):
    """Per-core input dicts."""
    f1 = np.asarray(inputs["features1"], np.float32)
    f2 = np.asarray(inputs["features2"], np.float32)
    f3 = np.asarray(inputs["features3"], np.float32)
    SLOTS = cfg.SLOTS
    node_core, node_pos = pre["node_core"], pre["node_pos"]

    import ml_dtypes

    BF = ml_dtypes.bfloat16

    def pack_T(feat):
        D = feat.shape[1]
        o = np.zeros((NCORES, D, SLOTS), BF)
        o[node_core, :, node_pos] = feat.astype(BF)
        return o

    X1, X2, X3 = pack_T(f1), pack_T(f2), pack_T(f3)

    def bias_fold(b):  # [256] -> [128, 2]
        return np.ascontiguousarray(np.asarray(b, np.float32).reshape(2, P).T)

    beta = (
        A1 * np.asarray(inputs["b1_1"])
        + A2 * np.asarray(inputs["b2_1"])
        + A3 * np.asarray(inputs["b3_1"])
    ).astype(np.float32)
    beta_rep = np.ascontiguousarray(np.broadcast_to(beta, (P, C)))

    common = {
        "w1": np.asarray(inputs["w1_0"], np.float32).astype(BF),
        "w2": np.asarray(inputs["w2_0"], np.float32).astype(BF),
        "w3": np.asarray(inputs["w3_0"], np.float32).astype(BF),
        "b1": bias_fold(inputs["b1_0"]),
        "b2": bias_fold(inputs["b2_0"]),
        "b3": bias_fold(inputs["b3_0"]),
        "w2p1": (A1 * np.asarray(inputs["w1_1"], np.float32)).astype(BF),
        "w2p2": (A2 * np.asarray(inputs["w2_1"], np.float32)).astype(BF),
        "w2p3": (A3 * np.asarray(inputs["w3_1"], np.float32)).astype(BF),
        "beta": beta_rep,
    }
    in_maps = []
    for c in range(NCORES):
        m = dict(common)
        m["x1t"] = np.ascontiguousarray(X1[c])
        m["x2t"] = np.ascontiguousarray(X2[c])
        m["x3t"] = np.ascontiguousarray(X3[c])
        m["idxw"] = pre["idxw"][c]
        m["slott"] = pre["slott"][c]
        for k in ("nfull", "n01", "avec", "avlast"):
            m[k] = pre["packs"][k][c]
        in_maps.append(m)
    return in_maps


_CACHE = {}


def run(inputs, cfg, **spmd_kwargs):
    pre_key = "pre"
    if pre_key not in _CACHE:
        _CACHE[pre_key] = preprocess(
            cfg,
            np.asarray(inputs["edge_src"]),
            np.asarray(inputs["edge_dst"]),
            np.asarray(inputs["norm"]),
        )
        _CACHE["nc"] = build(cfg)
    pre = _CACHE[pre_key]
    nc = _CACHE["nc"]
    in_maps = make_in_maps(cfg, pre, inputs)
    res = run_bass_kernel_spmd(nc, in_maps, core_ids=list(range(NCORES)), **spmd_kwargs)
    full = np.concatenate([res.results[c]["out"] for c in range(NCORES)], axis=0)
    out = full[pre["out_row"]]
    return out.astype(np.float32), res


def kernel(**inputs):
    cfg = Cfg(n_nodes=100000, n_edges=3200000, k_steps=10)
    out, _ = run(inputs, cfg)
    return out
